# revision 1
# baseline (speedup 1.0000x reference)
"""Trainium2 Bass kernel for nn_Attention_8143257993917.

Multi-head attention (packed QKV + RoPE + additive bias + softmax + head_mask
+ o_proj), B=4, S=2048, D=1024, H=16 heads, fp32 I/O.

Sharding: 8 cores = 4 batches x 2 head-groups (tensor-parallel over heads).
Core c handles batch b = c // 2 and heads g*8..g*8+8 with g = c % 2.
Each core computes a partial output (its heads' contribution through o_proj);
the host sums the two partials per batch and adds o_b.

Device-side design (per core, fast mode):
- Everything runs in "transposed" feature-major layouts so the big score /
  probability matrices never need an on-chip transpose:
    Q_T, K_T: [f, t] (f = head*64+d on partitions): out[f,t] = wT[d,f].T @ hT.
    RoPE: q' = (q + bq) * cos + (rot(q) + rot(bq)) * sin, where the rotated
      branch comes from a SECOND projection with host-prerotated weights
      (rotate_half is a row permutation+sign of W, so it folds into weights).
    V: [t, f] natural layout, so V chunks [k=128, d=64] are directly the
      stationary operand of the PV matmul. A ones-column appended to V makes
      the PV matmul also produce the softmax denominators (row 64 of ctx).
    scores S_T[k, q] = K_T_chunk.T @ Q_T (contraction d=64), fp16 operands,
      fp32 PSUM accumulate.
    bias: exp(S+b) = exp(S)*exp(b); exp(bias) is precomputed on the host in
      fp16 and applied as one elementwise multiply on VectorE (removes 512
      identity-matmul bias adds from the PE).
    exp on ScalarE (PSUM -> SBUF) with a constant -12 shift (softmax is
      shift-invariant; keeps exp outputs inside fp16 range).
    PV is software-pipelined one k-chunk behind scores/exp/mult so the PE
      never waits on the current chunk's ScalarE/VectorE results.
    softmax denominators: exact VectorE reciprocal on a [32, NQH/32] reshape
      (via a small DRAM round-trip on the gpsimd DMA queues, which also
      broadcasts 1/r across 64 partitions); one TT multiply normalizes ctx
      and moves it PSUM -> SBUF.
    head_mask is folded into the V projection weights/bias on the host.
    o_proj: out_T[o, t] = sum_f o_wT[f, o] * ctx_T[f, t], fp16 operands.
  Matmul dtype is fp16 rather than bf16: same PE throughput, ~8x lower
  quantization error (all value ranges verified to fit fp16 comfortably).
  fp32 matmuls on TRN2 lower to LOW_HIGH double-pass + 2 cycles/column
  streaming (~5x slower than fp16), hence the fp16 datapath with fp32
  accumulation; measured end-to-end relative error vs the fp32 reference
  is ~1.3e-3.
"""

import sys

sys.path.insert(0, "/opt/trn_rl_repo")

import numpy as np

_CACHE = {}

H = 16
HPC = 8  # heads per core
G = 2  # head groups


def build_nc(S=2048, D=1024, fast=True):
    """Build + compile the per-core Bass program (same program on all cores)."""
    import concourse.bass as bass
    from concourse import bacc
    import concourse.mybir as mybir
    import concourse.tile as tile
    from concourse.masks import make_identity
    from concourse.tile_rust import add_dep_helper

    F32 = mybir.dt.float32
    BF16 = mybir.dt.bfloat16
    F16 = mybir.dt.float16
    MT = F16 if fast else F32      # matmul operand dtype
    AF = mybir.ActivationFunctionType

    P = 128
    DC = D // P          # d chunks (contraction for projections)
    KC = S // P          # k chunks (scores contraction)
    NQH = S // 2         # q-half size
    NQ = min(512, NQH)   # matmul free-dim chunk
    NQC = NQH // NQ      # chunks per q-half
    FPC = HPC * 64       # features per core (= 512)
    FT = FPC // P        # f-tiles per tensor (= 4)
    NT = min(512, S)     # phase C t-chunk
    TT4 = S // NT
    NTA = min(512, NQH)  # phase A t-chunk

    nc = bacc.Bacc("TRN2", target_bir_lowering=False, debug=False, num_devices=8)

    hT = nc.dram_tensor("hT", [D, S], MT, kind="ExternalInput")
    w4 = nc.dram_tensor("w4", [D, 4 * FPC], MT, kind="ExternalInput")
    b4 = nc.dram_tensor("b4", [4 * FPC], F32, kind="ExternalInput")
    wvT = nc.dram_tensor("wvT", [D, FPC], MT, kind="ExternalInput")
    bv = nc.dram_tensor("bv", [FPC], MT, kind="ExternalInput")
    cosr = nc.dram_tensor("cosr", [P, S], F32, kind="ExternalInput")
    sinr = nc.dram_tensor("sinr", [P, S], F32, kind="ExternalInput")
    if fast:
        expbT = nc.dram_tensor("expbT", [S, S], F16, kind="ExternalInput")
    else:
        biasT = nc.dram_tensor("biasT", [S, S], F32, kind="ExternalInput")
    owT = nc.dram_tensor("owT", [FPC, D], MT, kind="ExternalInput")
    outT = nc.dram_tensor("outT", [D, S], F32, kind="ExternalOutput")

    hT_r = hT.ap().rearrange("(o p) t -> p o t", p=P)
    w4_r = w4.ap().rearrange("(o p) f -> p o f", p=P)
    wv_r = wvT.ap().rearrange("(o p) f -> p o f", p=P)
    ow_r = owT.ap().rearrange("(o p) f -> p o f", p=P)
    b4_r = b4.ap().rearrange("(o p) -> p o", p=P)

    with tile.TileContext(nc) as tc:
        with (
            tc.tile_pool(name="cst", bufs=1) as cst,
            tc.tile_pool(name="pAB", bufs=1) as pAB,
            tc.tile_pool(name="dram", bufs=4, space="DRAM") as dpool,
        ):
            ident = cst.tile([P, P], F32)
            make_identity(nc, ident)
            ones1 = cst.tile([1, P], MT)
            nc.vector.memset(ones1[:], 1.0)
            b4_sb = cst.tile([P, 4 * FPC // P], F32)
            nc.sync.dma_start(b4_sb[:], b4_r)
            bv_sb = cst.tile([1, FPC], MT)
            eshift = cst.tile([P, 1], F32)
            nc.vector.memset(eshift[:], -12.0)
            nc.sync.dma_start(bv_sb[:], bv.ap()[None, :])

            # persistent phase A->B products
            qk_sb = pAB.tile([P, 2 * FT, S], MT)          # slots: Q ft 0..FT-1, K ft FT..2FT-1
            v_sb = pAB.tile([P, KC, HPC, 66], MT)          # col 64 = ones

            nc.vector.memset(v_sb[:, :, :, 64:65], 1.0)

            PSW = max(NQH, 512)  # psum tag width (fp32 elems per partition)

            # ---------------- Phase A: projections + rope ----------------
            with (
                tc.tile_pool(name="pA", bufs=1) as pA,
                tc.tile_pool(name="pAw", bufs=2) as pAw,
                tc.tile_pool(name="psA", bufs=2, space="PSUM") as ppsA,
            ):
                for half in range(2):
                    tsl = slice(half * NQH, (half + 1) * NQH)
                    h_sb = pA.tile([P, DC, NQH], MT, tag="hT", bufs=2)
                    nc.sync.dma_start(h_sb[:], hT_r[:, :, tsl])
                    cos_sb = pA.tile([P, NQH], F32, tag="cos", bufs=2)
                    nc.sync.dma_start(cos_sb[:], cosr.ap()[:, tsl])
                    sin_sb = pA.tile([P, NQH], F32, tag="sin", bufs=2)
                    nc.sync.dma_start(sin_sb[:], sinr.ap()[:, tsl])

                    # Q/K (+rotated twins) -> qk_sb
                    for qk in range(2):            # 0 = Q, 1 = K
                        for ft in range(FT):
                            fcol = qk * 2 * FPC + ft * P       # col of plain tensor in w4
                            frcol = fcol + FPC                 # col of rotated twin
                            wa = pAw.tile([P, DC, P], MT, tag="wA")
                            nc.sync.dma_start(wa[:], w4_r[:, :, fcol:fcol + P])
                            wb = pAw.tile([P, DC, P], MT, tag="wB")
                            nc.sync.dma_start(wb[:], w4_r[:, :, frcol:frcol + P])
                            bcol = (qk * 2 * FPC + ft * P) // P
                            brcol = bcol + FPC // P
                            for tq in range(NQH // NTA):
                                qsl = slice(tq * NTA, (tq + 1) * NTA)
                                pa = ppsA.tile([P, NTA], F32, tag="pa", name="pa")
                                pb = ppsA.tile([P, NTA], F32, tag="pb", name="pb")
                                for dc in range(DC):
                                    nc.tensor.matmul(pa[:], wa[:, dc], h_sb[:, dc, qsl],
                                                     start=(dc == 0), stop=(dc == DC - 1))
                                for dc in range(DC):
                                    nc.tensor.matmul(pb[:], wb[:, dc], h_sb[:, dc, qsl],
                                                     start=(dc == 0), stop=(dc == DC - 1))
                                tca = pAw.tile([P, NTA], F32, tag="tca")
                                nc.vector.scalar_tensor_tensor(
                                    tca[:], pa[:], b4_sb[:, bcol:bcol + 1], cos_sb[:, qsl],
                                    op0=mybir.AluOpType.add, op1=mybir.AluOpType.mult)
                                tcb = pAw.tile([P, NTA], F32, tag="tcb")
                                nc.vector.scalar_tensor_tensor(
                                    tcb[:], pb[:], b4_sb[:, brcol:brcol + 1], sin_sb[:, qsl],
                                    op0=mybir.AluOpType.add, op1=mybir.AluOpType.mult)
                                dst = qk_sb[:, qk * FT + ft, half * NQH + tq * NTA:
                                            half * NQH + (tq + 1) * NTA]
                                nc.vector.tensor_add(dst, tca[:], tcb[:])

                    # V for this half: t-tiles within half
                    wvs = pA.tile([P, DC, FPC], MT, tag="wV")
                    nc.sync.dma_start(wvs[:], wv_r)
                    for tt in range(NQH // P):
                        gt = half * (NQH // P) + tt            # global t-tile = k-chunk
                        pv = ppsA.tile([P, FPC], F32, tag="pv", name="pv")
                        for dc in range(DC):
                            nc.tensor.matmul(pv[:], h_sb[:, dc, tt * P:(tt + 1) * P],
                                             wvs[:, dc], start=(dc == 0), stop=False)
                        nc.tensor.matmul(pv[:], ones1[:], bv_sb[:], start=False, stop=True)
                        nc.vector.tensor_copy(v_sb[:, gt, :, 0:64], pv[:])

            with tc.tile_pool(name="pBC", bufs=1) as pBC:
                ctxT = pBC.tile([P, FT, S], MT)            # normalized ctx, f-major
                ow_sb = pBC.tile([P, FT, D], MT)
                nc.sync.dma_start(ow_sb[:], ow_r)

                # ---------------- Phase B: attention ----------------
                with (
                    tc.tile_pool(name="pB", bufs=2) as pB,
                    tc.tile_pool(name="psB", bufs=1, space="PSUM") as ppsB,
                ):
                    for hp in range(HPC // 2):
                        for qh in range(2):
                            qoff = qh * NQH
                            cps = []
                            for i in range(2):
                                ct = ppsB.tile([P, NQH], F32, tag=f"ctx{i}",
                                               name=f"ctx{i}")
                                cps.append(ct[:65, :])
                            prev_us = None
                            prev_kc = -1
                            for kc in range(KC):
                                if fast:
                                    eb_sb = pB.tile([P, NQH], F16, tag="bias", bufs=3)
                                    nc.sync.dma_start(
                                        eb_sb[:],
                                        expbT.ap()[kc * P:(kc + 1) * P,
                                                   qoff:qoff + NQH])
                                else:
                                    bias_sb = pB.tile([P, NQH], F32, tag="bias")
                                    nc.sync.dma_start(
                                        bias_sb[:],
                                        biasT.ap()[kc * P:(kc + 1) * P,
                                                   qoff:qoff + NQH])
                                psS = []
                                for hi in range(2):
                                    psS.append(ppsB.tile([P, NQH], F32,
                                                         tag=f"s{hi}", name="psS"))
                                # scores: h0/h1 adjacent for row-group overlap
                                prev_mm = None
                                for qc in range(NQC):
                                    csl = slice(qc * NQ, (qc + 1) * NQ)
                                    for hi in range(2):
                                        h = 2 * hp + hi
                                        base = 64 * (h % 2)
                                        ft = h // 2
                                        ksl = qk_sb[base:base + 64, FT + ft,
                                                    kc * P:(kc + 1) * P]
                                        qsl = qk_sb[base:base + 64, ft,
                                                    qoff + qc * NQ:
                                                    qoff + (qc + 1) * NQ]
                                        mm = nc.tensor.matmul(psS[hi][:, csl], ksl,
                                                              qsl, start=True,
                                                              stop=fast)
                                        if prev_mm is not None:
                                            add_dep_helper(
                                                mm.ins, prev_mm.ins, sync=False,
                                                reason="scores row-group pairing")
                                        prev_mm = mm
                                        if not fast:
                                            nc.tensor.matmul(psS[hi][:, csl],
                                                             ident[:],
                                                             bias_sb[:, csl],
                                                             start=False, stop=True)
                                us = []
                                for hi in range(2):
                                    u_sb = pB.tile([P, NQH], MT, tag=f"u{hi}")
                                    if fast:
                                        nc.scalar.activation(u_sb[:], psS[hi][:],
                                                             AF.Exp, bias=eshift[:])
                                        u2 = pB.tile([P, NQH], F16, tag=f"u2{hi}")
                                        nc.vector.tensor_mul(u2[:], u_sb[:],
                                                             eb_sb[:])
                                        us.append(u2)
                                    else:
                                        nc.scalar.activation(u_sb[:], psS[hi][:],
                                                             AF.Exp)
                                        us.append(u_sb)
                                # software-pipeline: PV lags one kc so PE never
                                # waits on this cycle's exp/mult
                                if prev_us is not None:
                                    for qc in range(NQC):
                                        csl = slice(qc * NQ, (qc + 1) * NQ)
                                        for hi in range(2):
                                            h = 2 * hp + hi
                                            nc.tensor.matmul(
                                                cps[hi][:, csl],
                                                v_sb[:, prev_kc, h, 0:65],
                                                prev_us[hi][:, csl],
                                                start=(prev_kc == 0), stop=False)
                                prev_us, prev_kc = us, kc
                            for qc in range(NQC):
                                csl = slice(qc * NQ, (qc + 1) * NQ)
                                for hi in range(2):
                                    h = 2 * hp + hi
                                    nc.tensor.matmul(cps[hi][:, csl],
                                                     v_sb[:, prev_kc, h, 0:65],
                                                     prev_us[hi][:, csl],
                                                     start=False, stop=True)
                            # finalize: evacuate ctx PSUM -> SBUF immediately
                            # (frees the ctx banks for the next iteration's PV
                            # without waiting for the reciprocal chain), then
                            # normalize entirely from SBUF.
                            cus = []
                            for hi in range(2):
                                cu = pB.tile([65, NQH], F32, tag=f"cu{hi}")
                                nc.scalar.copy(cu[:], cps[hi][:])
                                cus.append(cu)
                            rscrs, rsqs, rrecs, rscr2s, rbs = [], [], [], [], []
                            for hi in range(2):
                                rscr = dpool.tile([NQH], F32)
                                nc.gpsimd.dma_start(rscr[None, :],
                                                    cus[hi][64:65, :])
                                rscrs.append(rscr)
                            for hi in range(2):
                                rsq = pB.tile([32, NQH // 32], F32, tag=f"rsq{hi}")
                                nc.gpsimd.dma_start(
                                    rsq[:], rscrs[hi].rearrange("(a b) -> a b", a=32))
                                rsqs.append(rsq)
                            for hi in range(2):
                                rrec = pB.tile([32, NQH // 32], F32, tag=f"rrec{hi}")
                                nc.vector.reciprocal(rrec[:], rsqs[hi][:])
                                rrecs.append(rrec)
                            for hi in range(2):
                                rscr2 = dpool.tile([NQH], F32)
                                nc.gpsimd.dma_start(
                                    rscr2.rearrange("(a b) -> a b", a=32), rrecs[hi][:])
                                rscr2s.append(rscr2)
                            for hi in range(2):
                                rb = pB.tile([64, NQH], F32, tag=f"rb{hi}")
                                nc.gpsimd.dma_start(rb[:],
                                                    rscr2s[hi].partition_broadcast(64))
                                rbs.append(rb)
                            for hi in range(2):
                                h = 2 * hp + hi
                                base = 64 * (h % 2)
                                ft = h // 2
                                nc.vector.tensor_mul(
                                    ctxT[base:base + 64, ft, qoff:qoff + NQH],
                                    cus[hi][0:64, :], rbs[hi][:])

                # ---------------- Phase C: output projection ----------------
                with (
                    tc.tile_pool(name="pC", bufs=2) as pC,
                    tc.tile_pool(name="psC", bufs=2, space="PSUM") as ppsC,
                ):
                    for ot in range(D // P):
                        for tq in range(TT4):
                            tsl = slice(tq * NT, (tq + 1) * NT)
                            po = ppsC.tile([P, NT], F32, tag="po", name="po")
                            for fc in range(FT):
                                nc.tensor.matmul(po[:],
                                                 ow_sb[:, fc, ot * P:(ot + 1) * P],
                                                 ctxT[:, fc, tsl],
                                                 start=(fc == 0), stop=(fc == FT - 1))
                            o_sb = pC.tile([P, NT], F32, tag="oT")
                            nc.scalar.copy(o_sb[:], po[:])
                            nc.sync.dma_start(outT.ap()[ot * P:(ot + 1) * P, tsl],
                                              o_sb[:])

    nc.compile()
    return nc


def make_core_inputs(hidden_states, attention_bias, rope_cos, rope_sin, head_mask,
                     qkv_w, qkv_b, o_w, S=2048, D=1024, fast=True):
    """Host-side sharding + layout preparation. Returns list of 8 input dicts."""
    f32 = np.float32
    mt = np.float16 if fast else np.float32
    f16 = np.float16
    hidden_states = np.asarray(hidden_states, f32)
    attention_bias = np.asarray(attention_bias, f32)
    rope_cos = np.asarray(rope_cos, f32)
    rope_sin = np.asarray(rope_sin, f32)
    head_mask = np.asarray(head_mask, f32).reshape(-1)
    qkv_w = np.asarray(qkv_w, f32)
    qkv_b = np.asarray(qkv_b, f32)
    o_w = np.asarray(o_w, f32)

    B = hidden_states.shape[0]
    FPC = HPC * 64
    F = H * 64  # qkv feature dim (row-section size of qkv_w)

    def rot_rows(w):
        # rows indexed by f = hl*64 + d; rot(q)[d] = -q[d+32] (d<32) else q[d-32]
        w = w.reshape(HPC, 64, -1) if w.ndim == 2 else w.reshape(HPC, 64)
        lo, hi = w[:, 0:32], w[:, 32:64]
        out = np.concatenate([-hi, lo], axis=1)
        return out.reshape(HPC * 64, -1) if out.ndim == 3 else out.reshape(HPC * 64)

    cos_t = rope_cos[0, :, 0, :].T.astype(f32)     # [64, S]
    sin_t = rope_sin[0, :, 0, :].T.astype(f32)
    cosr = np.concatenate([cos_t, cos_t], axis=0)  # [128, S]
    sinr = np.concatenate([sin_t, sin_t], axis=0)

    in_maps = []
    for c in range(8):
        b, g = divmod(c, G)
        fs = slice(g * FPC, (g + 1) * FPC)
        wq = qkv_w[F * 0:F * 1][fs]
        wk = qkv_w[F * 1:F * 2][fs]
        wv = qkv_w[F * 2:F * 3][fs].copy()
        bq = qkv_b[F * 0:F * 1][fs]
        bk = qkv_b[F * 1:F * 2][fs]
        bvv = qkv_b[F * 2:F * 3][fs].copy()
        mask = head_mask[g * HPC:(g + 1) * HPC]
        wv *= np.repeat(mask, 64)[:, None]
        bvv *= np.repeat(mask, 64)
        wqr, bqr = rot_rows(wq), rot_rows(bq)
        wkr, bkr = rot_rows(wk), rot_rows(bk)
        w4 = np.concatenate([wq.T, wqr.T, wk.T, wkr.T], axis=1)  # [D, 4*FPC]
        b4 = np.concatenate([bq, bqr, bk, bkr])
        bT = np.ascontiguousarray(attention_bias[b, 0].T)
        m = {
            "hT": np.ascontiguousarray(hidden_states[b].T).astype(mt),
            "w4": np.ascontiguousarray(w4).astype(mt),
            "b4": np.ascontiguousarray(b4),
            "wvT": np.ascontiguousarray(wv.T).astype(mt),
            "bv": np.ascontiguousarray(bvv).astype(mt),
            "cosr": np.ascontiguousarray(cosr),
            "sinr": np.ascontiguousarray(sinr),
            "owT": np.ascontiguousarray(o_w[:, g * FPC:(g + 1) * FPC].T).astype(mt),
        }
        if fast:
            m["expbT"] = np.exp(bT).astype(f16)
        else:
            m["biasT"] = bT
        in_maps.append(m)
    return in_maps


def kernel(hidden_states, attention_bias, rope_cos, rope_sin, head_mask,
           qkv_w, qkv_b, o_w, o_b, **_unused):
    from concourse.bass_utils import run_bass_kernel_spmd

    B, S, D = hidden_states.shape
    fast = _CACHE.get("fast", True)
    if "nc" not in _CACHE:
        _CACHE["nc"] = build_nc(S=S, D=D, fast=fast)
    nc = _CACHE["nc"]

    in_maps = make_core_inputs(hidden_states, attention_bias, rope_cos, rope_sin,
                               head_mask, qkv_w, qkv_b, o_w, S=S, D=D, fast=fast)
    res = run_bass_kernel_spmd(nc, in_maps, list(range(8)))
    _CACHE["last_results"] = res

    o_b = np.asarray(o_b, np.float32)
    out = np.empty((B, S, D), np.float32)
    for b in range(B):
        acc = res.results[2 * b]["outT"].T + res.results[2 * b + 1]["outT"].T
        out[b] = acc + o_b[None, :]
    return out



# revision 7
# speedup vs baseline: 1.1633x; 1.1633x over previous
"""Trainium2 Bass kernel for nn_Attention_8143257993917.

Multi-head attention (packed QKV + RoPE + additive bias + softmax + head_mask
+ o_proj), B=4, S=2048, D=1024, H=16 heads, fp32 I/O.

Sharding: 8 cores = 4 batches x 2 head-groups (tensor-parallel over heads).
Core c handles batch b = c // 2 and heads g*8..g*8+8 with g = c % 2.
Each core computes a partial output (its heads' contribution through o_proj);
the host sums the two partials per batch and adds o_b.

v2 design (per core), all feature-major layouts, fp16 matmul datapath with
fp32 PSUM accumulation:
  Phase A (projections + rope):
    Q_T/K_T [f, t]: psum = w_tile.T @ hT, evacuated on ScalarE with the bias
    add folded in (activation Copy + per-partition bias), then
    q' = qpb*cos + rot(qpb)*sin where rot(qpb) comes from a single [128,128]
    signed-permutation matmul (rotate_half folds into a constant matrix and
    the bias is already inside qpb). N=1024 moving operands.
    V [t, f] via stationary-h matmuls (so V chunks are directly the PV
    stationary); ones-row matmul adds the (head_mask-folded) V bias; a
    ones-column appended to V makes PV also produce softmax denominators.
  Phase B (attention), loop qh(2) outer x hp(4), per-kc software pipeline:
    scores sc_hi[k,q] = K_chunk.T @ Q (row-tiled 64x128: h-even on array
    rows 0-63, h-odd on 64-127), exp on ScalarE (PSUM->SBUF, constant -12
    shift), u2 = exp(s)*exp(bias) on DVE (exp(bias) precomputed on host,
    fp16, loaded once per qh half), PV lags one kc. Split per-head exp
    instructions let head h0's next-kc scores matmul run while h1's exp
    occupies ScalarE, keeping ScalarE (the throughput floor: ~33.6M exp
    elems/core, ScalarE-only) saturated. PSUM: 2 scores + 2 ctx tiles = 8
    banks exactly.
    Denominator reciprocal via DRAM round-trip reshape + partition
    broadcast on the gpsimd DMA queues (as baseline).
  Phase C: o_proj out_T[o,t] = sum_fc ow.T @ ctxT, fp16 output (host sums
    partials in fp32).
"""

import sys

sys.path.insert(0, "/opt/trn_rl_repo")

import numpy as np

_CACHE = {}

H = 16
HPC = 8  # heads per core
G = 2  # head groups


def build_nc(S=2048, D=1024):
    import concourse.bass as bass
    from concourse import bacc
    import concourse.mybir as mybir
    import concourse.tile as tile

    F32 = mybir.dt.float32
    F16 = mybir.dt.float16
    AF = mybir.ActivationFunctionType
    ADD = mybir.AluOpType.add
    MULT = mybir.AluOpType.mult

    P = 128
    DC = D // P           # contraction chunks for projections (8)
    KC = S // P           # k chunks (16)
    NQH = S // 2          # q half (1024)
    NQ = 512              # matmul moving free-dim max
    FPC = HPC * 64        # features per core (512)
    FT = FPC // P         # f-tiles per tensor (4)

    nc = bacc.Bacc("TRN2", target_bir_lowering=False, debug=False, num_devices=8)

    hT = nc.dram_tensor("hT", [D, S], F16, kind="ExternalInput")
    w4 = nc.dram_tensor("w4", [D, 2 * FPC], F16, kind="ExternalInput")
    b4 = nc.dram_tensor("b4", [2 * FPC], F32, kind="ExternalInput")
    wvT = nc.dram_tensor("wvT", [D, FPC], F16, kind="ExternalInput")
    bv = nc.dram_tensor("bv", [FPC], F16, kind="ExternalInput")
    cosr = nc.dram_tensor("cosr", [P, S], F16, kind="ExternalInput")
    sinr = nc.dram_tensor("sinr", [P, S], F16, kind="ExternalInput")
    permM = nc.dram_tensor("permM", [P, P], F16, kind="ExternalInput")
    expbT = nc.dram_tensor("expbT", [S, S], F16, kind="ExternalInput")
    owT = nc.dram_tensor("owT", [FPC, D], F16, kind="ExternalInput")
    outT = nc.dram_tensor("outT", [D, S], F16, kind="ExternalOutput")

    hT_r = hT.ap().rearrange("(o p) t -> p o t", p=P)
    w4_r = w4.ap().rearrange("(o p) f -> p o f", p=P)
    wv_r = wvT.ap().rearrange("(o p) f -> p o f", p=P)
    ow_r = owT.ap().rearrange("(o p) f -> p o f", p=P)
    b4_r = b4.ap().rearrange("(o p) -> p o", p=P)
    eb_r = expbT.ap().rearrange("(kc p) q -> p kc q", p=P)

    with tile.TileContext(nc) as tc:
        with (
            tc.tile_pool(name="cst", bufs=1) as cst,
            tc.tile_pool(name="pAB", bufs=1) as pAB,
            tc.tile_pool(name="dram", bufs=4, space="DRAM") as dpool,
        ):
            ones1 = cst.tile([1, P], F16)
            nc.vector.memset(ones1[:], 1.0)
            b4_sb = cst.tile([P, 2 * FPC // P], F32)
            nc.sync.dma_start(b4_sb[:], b4_r)
            bv_sb = cst.tile([1, FPC], F16)
            nc.sync.dma_start(bv_sb[:], bv.ap()[None, :])
            eshift = cst.tile([P, 1], F32)
            nc.vector.memset(eshift[:], -12.0)
            permM_sb = cst.tile([P, P], F16)
            nc.sync.dma_start(permM_sb[:], permM.ap())

            # persistent products of phase A
            qk_sb = pAB.tile([P, 2 * FT, S], F16)      # Q ft 0..3, K ft 4..7
            v_sb = pAB.tile([P, KC, HPC, 66], F16)     # col 64 = ones
            nc.vector.memset(v_sb[:, :, :, 64:65], 1.0)

            ctxT = pAB.tile([P, FT, S], F16)
            ow_sb = pAB.tile([P, FT, D], F16)
            nc.sync.dma_start(ow_sb[:], ow_r)

            # ---------------- Phase A ----------------
            with (
                tc.tile_pool(name="pA", bufs=1) as pA,
                tc.tile_pool(name="pAw", bufs=2) as pAw,
                tc.tile_pool(name="psA", bufs=1, space="PSUM") as psA,
            ):
                h_sb = pA.tile([P, DC, S], F16)
                for dc in range(DC):
                    nc.sync.dma_start(h_sb[:, dc], hT_r[:, dc])
                cos_sb = pA.tile([P, S], F16)
                nc.sync.dma_start(cos_sb[:], cosr.ap())
                sin_sb = pA.tile([P, S], F16)
                nc.sync.dma_start(sin_sb[:], sinr.ap())
                wv_sb = pA.tile([P, DC, FPC], F16)
                nc.sync.dma_start(wv_sb[:], wv_r)

                # Q/K projection + rope, one 128-feature tile at a time
                for j in range(2 * FT):                # 0..3 Q, 4..7 K
                    wa = pAw.tile([P, DC, P], F16, tag="wA")
                    nc.sync.dma_start(wa[:], w4_r[:, :, j * P:(j + 1) * P])
                    pas = []
                    for half in range(2):
                        pa = psA.tile([P, NQH], F32, tag=f"pa{half}",
                                      name=f"pa{half}")
                        pas.append(pa)
                    for dc in range(DC):
                        for half in range(2):
                            for tq in range(2):
                                tsl = slice(half * NQH + tq * NQ,
                                            half * NQH + (tq + 1) * NQ)
                                nc.tensor.matmul(pas[half][:, tq * NQ:(tq + 1) * NQ],
                                                 wa[:, dc], h_sb[:, dc, tsl],
                                                 start=(dc == 0), stop=(dc == DC - 1))
                    for half in range(2):
                        tsl = slice(half * NQH, (half + 1) * NQH)
                        qpb = pAw.tile([P, NQH], F16, tag=f"qpb{half}")
                        nc.scalar.activation(qpb[:], pas[half][:], AF.Identity,
                                             bias=b4_sb[:, j:j + 1])
                        pr = psA.tile([P, NQH], F32, tag="pr", name="pr")
                        for tq in range(2):
                            nc.tensor.matmul(pr[:, tq * NQ:(tq + 1) * NQ],
                                             permM_sb[:],
                                             qpb[:, tq * NQ:(tq + 1) * NQ],
                                             start=True, stop=True)
                        t1 = pAw.tile([P, NQH], F16, tag=f"t1{half}")
                        nc.vector.tensor_mul(t1[:], qpb[:], cos_sb[:, tsl])
                        t2 = pAw.tile([P, NQH], F16, tag=f"t2{half}")
                        nc.vector.tensor_mul(t2[:], pr[:], sin_sb[:, tsl])
                        nc.vector.tensor_add(qk_sb[:, j, tsl], t1[:], t2[:])

                # V projection: [t, f] via stationary-h matmuls
                for tt in range(KC):
                    pv = psA.tile([P, FPC], F32, tag="pv", name="pv")
                    for dc in range(DC):
                        nc.tensor.matmul(pv[:], h_sb[:, dc, tt * P:(tt + 1) * P],
                                         wv_sb[:, dc], start=(dc == 0), stop=False)
                    nc.tensor.matmul(pv[:], ones1[:], bv_sb[:],
                                     start=False, stop=True)
                    nc.vector.tensor_copy(v_sb[:, tt, :, 0:64], pv[:])

            # ---------------- Phase B ----------------
            with (
                tc.tile_pool(name="peb", bufs=1) as peb,
                tc.tile_pool(name="pB", bufs=2) as pB,
                tc.tile_pool(name="psB", bufs=1, space="PSUM") as psB,
            ):
                for qh in range(2):
                    qsl = slice(qh * NQH, (qh + 1) * NQH)
                    eb = peb.tile([P, KC, NQH], F16, tag="eb", bufs=2)
                    nc.sync.dma_start(eb[:], eb_r[:, :, qsl])
                    for hp in range(FT):
                        ft = hp
                        scs, cts = [], []
                        for hi in range(2):
                            sc = psB.tile([P, NQH], F32, tag=f"sc{hi}",
                                          name=f"sc{hi}")
                            scs.append(sc)
                            ct = psB.tile([P, NQH], F32, tag=f"ct{hi}",
                                          name=f"ct{hi}")
                            cts.append(ct[:65, :])
                        prev = [None, None]
                        for kc in range(KC):
                            cur = [None, None]
                            for hi in range(2):
                                base = 64 * hi
                                h = 2 * hp + hi
                                ksl = qk_sb[base:base + 64, FT + ft,
                                            kc * P:(kc + 1) * P]
                                qq = qk_sb[base:base + 64, ft, qsl]
                                for qc in range(2):
                                    csl = slice(qc * NQ, (qc + 1) * NQ)
                                    nc.tensor.matmul(scs[hi][:, csl], ksl,
                                                     qq[:, csl],
                                                     start=True, stop=True)
                                u = pB.tile([P, NQH], F16, tag=f"u{hi}")
                                nc.scalar.activation(u[:], scs[hi][:], AF.Exp,
                                                     bias=eshift[:])
                                u2 = pB.tile([P, NQH], F16, tag=f"u2{hi}")
                                nc.vector.tensor_mul(u2[:], u[:], eb[:, kc])
                                cur[hi] = u2
                                if prev[hi] is not None:
                                    for qc in range(2):
                                        csl = slice(qc * NQ, (qc + 1) * NQ)
                                        nc.tensor.matmul(
                                            cts[hi][:, csl],
                                            v_sb[:, kc - 1, h, 0:65],
                                            prev[hi][:, csl],
                                            start=(kc == 1), stop=False)
                            prev = cur
                        for hi in range(2):
                            h = 2 * hp + hi
                            for qc in range(2):
                                csl = slice(qc * NQ, (qc + 1) * NQ)
                                nc.tensor.matmul(cts[hi][:, csl],
                                                 v_sb[:, KC - 1, h, 0:65],
                                                 prev[hi][:, csl],
                                                 start=False, stop=True)
                        # finalize: evacuate ctx, reciprocal via DRAM roundtrip
                        cus = []
                        for hi in range(2):
                            cu = pB.tile([65, NQH], F32, tag=f"cu{hi}")
                            nc.vector.tensor_copy(cu[:], cts[hi])
                            cus.append(cu)
                        rscrs, rsqs, rrecs, rscr2s, rbs = [], [], [], [], []
                        for hi in range(2):
                            rscr = dpool.tile([NQH], F32)
                            nc.gpsimd.dma_start(rscr[None, :], cus[hi][64:65, :])
                            rscrs.append(rscr)
                        for hi in range(2):
                            rsq = pB.tile([32, NQH // 32], F32, tag=f"rsq{hi}")
                            nc.gpsimd.dma_start(
                                rsq[:], rscrs[hi].rearrange("(a b) -> a b", a=32))
                            rsqs.append(rsq)
                        for hi in range(2):
                            rrec = pB.tile([32, NQH // 32], F32, tag=f"rrec{hi}")
                            nc.vector.reciprocal(rrec[:], rsqs[hi][:])
                            rrecs.append(rrec)
                        for hi in range(2):
                            rscr2 = dpool.tile([NQH], F32)
                            nc.gpsimd.dma_start(
                                rscr2.rearrange("(a b) -> a b", a=32), rrecs[hi][:])
                            rscr2s.append(rscr2)
                        for hi in range(2):
                            rb = pB.tile([64, NQH], F32, tag=f"rb{hi}")
                            nc.gpsimd.dma_start(rb[:],
                                                rscr2s[hi].partition_broadcast(64))
                            rbs.append(rb)
                        for hi in range(2):
                            base = 64 * hi
                            nc.vector.tensor_mul(ctxT[base:base + 64, ft, qsl],
                                                 cus[hi][0:64, :], rbs[hi][:])

            # ---------------- Phase C ----------------
            with (
                tc.tile_pool(name="pC", bufs=3) as pC,
                tc.tile_pool(name="psC", bufs=2, space="PSUM") as psC,
            ):
                NT = NQ
                for ot in range(D // P):
                    for tq in range(S // NT):
                        tsl = slice(tq * NT, (tq + 1) * NT)
                        po = psC.tile([P, NT], F32, tag="po", name="po")
                        for fc in range(FT):
                            nc.tensor.matmul(po[:],
                                             ow_sb[:, fc, ot * P:(ot + 1) * P],
                                             ctxT[:, fc, tsl],
                                             start=(fc == 0), stop=(fc == FT - 1))
                        o_sb = pC.tile([P, NT], F16, tag="oT")
                        nc.scalar.copy(o_sb[:], po[:])
                        nc.sync.dma_start(outT.ap()[ot * P:(ot + 1) * P, tsl],
                                          o_sb[:])

    nc.compile()
    return nc


def make_core_inputs(hidden_states, attention_bias, rope_cos, rope_sin, head_mask,
                     qkv_w, qkv_b, o_w, S=2048, D=1024):
    """Host-side sharding + layout preparation. Returns list of 8 input dicts."""
    f32 = np.float32
    f16 = np.float16
    hidden_states = np.asarray(hidden_states, f32)
    attention_bias = np.asarray(attention_bias, f32)
    rope_cos = np.asarray(rope_cos, f32)
    rope_sin = np.asarray(rope_sin, f32)
    head_mask = np.asarray(head_mask, f32).reshape(-1)
    qkv_w = np.asarray(qkv_w, f32)
    qkv_b = np.asarray(qkv_b, f32)
    o_w = np.asarray(o_w, f32)

    B = hidden_states.shape[0]
    FPC = HPC * 64
    F = H * 64  # qkv feature dim (row-section size of qkv_w)

    cos_t = rope_cos[0, :, 0, :].T.astype(f32)     # [64, S]
    sin_t = rope_sin[0, :, 0, :].T.astype(f32)
    cosr = np.concatenate([cos_t, cos_t], axis=0)  # [128, S]
    sinr = np.concatenate([sin_t, sin_t], axis=0)

    # rotate_half as a signed permutation: out[c] = -in[c+32] (c%64<32),
    # +in[c-32] (c%64>=32); per 64-row head block, two blocks per 128.
    permM = np.zeros((128, 128), f32)
    for blk in (0, 64):
        for c in range(32):
            permM[blk + c + 32, blk + c] = -1.0
        for c in range(32, 64):
            permM[blk + c - 32, blk + c] = 1.0

    in_maps = []
    for c in range(8):
        b, g = divmod(c, G)
        fs = slice(g * FPC, (g + 1) * FPC)
        wq = qkv_w[F * 0:F * 1][fs]
        wk = qkv_w[F * 1:F * 2][fs]
        wv = qkv_w[F * 2:F * 3][fs].copy()
        bq = qkv_b[F * 0:F * 1][fs]
        bk = qkv_b[F * 1:F * 2][fs]
        bvv = qkv_b[F * 2:F * 3][fs].copy()
        mask = head_mask[g * HPC:(g + 1) * HPC]
        wv *= np.repeat(mask, 64)[:, None]
        bvv *= np.repeat(mask, 64)
        w4 = np.concatenate([wq.T, wk.T], axis=1)      # [D, 2*FPC]
        b4 = np.concatenate([bq, bk])
        bT = np.ascontiguousarray(attention_bias[b, 0].T)
        m = {
            "hT": np.ascontiguousarray(hidden_states[b].T).astype(f16),
            "w4": np.ascontiguousarray(w4).astype(f16),
            "b4": np.ascontiguousarray(b4),
            "wvT": np.ascontiguousarray(wv.T).astype(f16),
            "bv": np.ascontiguousarray(bvv).astype(f16),
            "cosr": np.ascontiguousarray(cosr).astype(f16),
            "sinr": np.ascontiguousarray(sinr).astype(f16),
            "permM": np.ascontiguousarray(permM).astype(f16),
            "expbT": np.exp(bT).astype(f16),
            "owT": np.ascontiguousarray(o_w[:, g * FPC:(g + 1) * FPC].T).astype(f16),
        }
        in_maps.append(m)
    return in_maps


def kernel(hidden_states, attention_bias, rope_cos, rope_sin, head_mask,
           qkv_w, qkv_b, o_w, o_b, **_unused):
    from concourse.bass_utils import run_bass_kernel_spmd

    B, S, D = hidden_states.shape
    if "nc" not in _CACHE:
        _CACHE["nc"] = build_nc(S=S, D=D)
    nc = _CACHE["nc"]

    in_maps = make_core_inputs(hidden_states, attention_bias, rope_cos, rope_sin,
                               head_mask, qkv_w, qkv_b, o_w, S=S, D=D)
    res = run_bass_kernel_spmd(nc, in_maps, list(range(8)))
    _CACHE["last_results"] = res

    o_b = np.asarray(o_b, np.float32)
    out = np.empty((B, S, D), np.float32)
    for b in range(B):
        acc = (res.results[2 * b]["outT"].astype(np.float32).T
               + res.results[2 * b + 1]["outT"].astype(np.float32).T)
        out[b] = acc + o_b[None, :]
    return out


# revision 8
# speedup vs baseline: 1.2935x; 1.1119x over previous
"""Trainium2 Bass kernel for nn_Attention_8143257993917.

Multi-head attention (packed QKV + RoPE + additive bias + softmax + head_mask
+ o_proj), B=4, S=2048, D=1024, H=16 heads, fp32 I/O.

Sharding: 8 cores = 4 batches x 2 head-groups (tensor-parallel over heads).
Core c handles batch b = c // 2 and heads g*8..g*8+8 with g = c % 2.
Each core computes a partial output (its heads' contribution through o_proj);
the host sums the two partials per batch and adds o_b.

v2 design (per core), all feature-major layouts, fp16 matmul datapath with
fp32 PSUM accumulation:
  Phase A (projections + rope):
    Q_T/K_T [f, t]: psum = w_tile.T @ hT, evacuated on ScalarE with the bias
    add folded in (activation Copy + per-partition bias), then
    q' = qpb*cos + rot(qpb)*sin where rot(qpb) comes from a single [128,128]
    signed-permutation matmul (rotate_half folds into a constant matrix and
    the bias is already inside qpb). N=1024 moving operands.
    V [t, f] via stationary-h matmuls (so V chunks are directly the PV
    stationary); ones-row matmul adds the (head_mask-folded) V bias; a
    ones-column appended to V makes PV also produce softmax denominators.
  Phase B (attention), loop qh(2) outer x hp(4), per-kc software pipeline:
    scores sc_hi[k,q] = K_chunk.T @ Q (row-tiled 64x128: h-even on array
    rows 0-63, h-odd on 64-127), exp on ScalarE (PSUM->SBUF, constant -12
    shift), u2 = exp(s)*exp(bias) on DVE (exp(bias) precomputed on host,
    fp16, loaded once per qh half), PV lags one kc. Split per-head exp
    instructions let head h0's next-kc scores matmul run while h1's exp
    occupies ScalarE, keeping ScalarE (the throughput floor: ~33.6M exp
    elems/core, ScalarE-only) saturated. PSUM: 2 scores + 2 ctx tiles = 8
    banks exactly.
    Denominator reciprocal via DRAM round-trip reshape + partition
    broadcast on the gpsimd DMA queues (as baseline).
  Phase C: o_proj out_T[o,t] = sum_fc ow.T @ ctxT, fp16 output (host sums
    partials in fp32).
"""

import sys

sys.path.insert(0, "/opt/trn_rl_repo")

import numpy as np

_CACHE = {}

H = 16
HPC = 8  # heads per core
G = 2  # head groups


def build_nc(S=2048, D=1024):
    import concourse.bass as bass
    from concourse import bacc
    import concourse.mybir as mybir
    import concourse.tile as tile

    F32 = mybir.dt.float32
    F16 = mybir.dt.float16
    AF = mybir.ActivationFunctionType
    ADD = mybir.AluOpType.add
    MULT = mybir.AluOpType.mult

    P = 128
    DC = D // P           # contraction chunks for projections (8)
    KC = S // P           # k chunks (16)
    NQH = S // 2          # q half (1024)
    NQ = 512              # matmul moving free-dim max
    FPC = HPC * 64        # features per core (512)
    FT = FPC // P         # f-tiles per tensor (4)

    nc = bacc.Bacc("TRN2", target_bir_lowering=False, debug=False, num_devices=8)

    hT = nc.dram_tensor("hT", [D, S], F16, kind="ExternalInput")
    w4 = nc.dram_tensor("w4", [D, 2 * FPC], F16, kind="ExternalInput")
    b4 = nc.dram_tensor("b4", [2 * FPC], F32, kind="ExternalInput")
    wvT = nc.dram_tensor("wvT", [D, FPC], F16, kind="ExternalInput")
    bv = nc.dram_tensor("bv", [FPC], F16, kind="ExternalInput")
    cosr = nc.dram_tensor("cosr", [P, S], F16, kind="ExternalInput")
    sinr = nc.dram_tensor("sinr", [P, S], F16, kind="ExternalInput")
    permM = nc.dram_tensor("permM", [P, P], F16, kind="ExternalInput")
    expbT = nc.dram_tensor("expbT", [S, S], F16, kind="ExternalInput")
    owT = nc.dram_tensor("owT", [FPC, D], F16, kind="ExternalInput")
    outT = nc.dram_tensor("outT", [D, S], F16, kind="ExternalOutput")

    hT_r = hT.ap().rearrange("(o p) t -> p o t", p=P)
    w4_r = w4.ap().rearrange("(o p) f -> p o f", p=P)
    wv_r = wvT.ap().rearrange("(o p) f -> p o f", p=P)
    ow_r = owT.ap().rearrange("(o p) f -> p o f", p=P)
    b4_r = b4.ap().rearrange("(o p) -> p o", p=P)
    eb_r = expbT.ap().rearrange("(kc p) q -> p kc q", p=P)

    with tile.TileContext(nc) as tc:
        with (
            tc.tile_pool(name="cst", bufs=1) as cst,
            tc.tile_pool(name="pAB", bufs=1) as pAB,
            tc.tile_pool(name="dram", bufs=4, space="DRAM") as dpool,
        ):
            ones1 = cst.tile([1, P], F16)
            nc.vector.memset(ones1[:], 1.0)
            b4_sb = cst.tile([P, 2 * FPC // P], F32)
            nc.sync.dma_start(b4_sb[:], b4_r)
            bv_sb = cst.tile([1, FPC], F16)
            nc.sync.dma_start(bv_sb[:], bv.ap()[None, :])
            eshift = cst.tile([P, 1], F32)
            nc.vector.memset(eshift[:], -12.0)
            permM_sb = cst.tile([P, P], F16)
            nc.sync.dma_start(permM_sb[:], permM.ap())

            # persistent products of phase A
            qk_sb = pAB.tile([P, 2 * FT, S], F16)      # Q ft 0..3, K ft 4..7
            v_sb = pAB.tile([P, KC, HPC, 66], F16)     # col 64 = ones
            nc.vector.memset(v_sb[:, :, :, 64:65], 1.0)

            ctxT = pAB.tile([P, FT, S], F16)
            ow_sb = pAB.tile([P, FT, D], F16)
            nc.sync.dma_start(ow_sb[:], ow_r)

            # ---------------- Phase A ----------------
            with (
                tc.tile_pool(name="pA", bufs=1) as pA,
                tc.tile_pool(name="pAw", bufs=2) as pAw,
                tc.tile_pool(name="psA", bufs=1, space="PSUM") as psA,
            ):
                h_sb = pA.tile([P, DC, S], F16)
                for dc in range(DC):
                    nc.sync.dma_start(h_sb[:, dc], hT_r[:, dc])
                cos_sb = pA.tile([P, S], F16)
                nc.sync.dma_start(cos_sb[:], cosr.ap())
                sin_sb = pA.tile([P, S], F16)
                nc.sync.dma_start(sin_sb[:], sinr.ap())
                wv_sb = pA.tile([P, DC, FPC], F16)
                nc.sync.dma_start(wv_sb[:], wv_r)

                # Q/K projection + rope, one 128-feature tile at a time
                for j in range(2 * FT):                # 0..3 Q, 4..7 K
                    wa = pAw.tile([P, DC, P], F16, tag="wA")
                    nc.sync.dma_start(wa[:], w4_r[:, :, j * P:(j + 1) * P])
                    pas = []
                    for half in range(2):
                        pa = psA.tile([P, NQH], F32, tag=f"pa{half}",
                                      name=f"pa{half}")
                        pas.append(pa)
                    for dc in range(DC):
                        for half in range(2):
                            for tq in range(2):
                                tsl = slice(half * NQH + tq * NQ,
                                            half * NQH + (tq + 1) * NQ)
                                nc.tensor.matmul(pas[half][:, tq * NQ:(tq + 1) * NQ],
                                                 wa[:, dc], h_sb[:, dc, tsl],
                                                 start=(dc == 0), stop=(dc == DC - 1))
                    for half in range(2):
                        tsl = slice(half * NQH, (half + 1) * NQH)
                        qpb = pAw.tile([P, NQH], F16, tag=f"qpb{half}")
                        nc.scalar.activation(qpb[:], pas[half][:], AF.Identity,
                                             bias=b4_sb[:, j:j + 1])
                        pr = psA.tile([P, NQH], F32, tag="pr", name="pr")
                        for tq in range(2):
                            nc.tensor.matmul(pr[:, tq * NQ:(tq + 1) * NQ],
                                             permM_sb[:],
                                             qpb[:, tq * NQ:(tq + 1) * NQ],
                                             start=True, stop=True)
                        t1 = pAw.tile([P, NQH], F16, tag=f"t1{half}")
                        nc.vector.tensor_mul(t1[:], qpb[:], cos_sb[:, tsl])
                        t2 = pAw.tile([P, NQH], F16, tag=f"t2{half}")
                        nc.vector.tensor_mul(t2[:], pr[:], sin_sb[:, tsl])
                        nc.vector.tensor_add(qk_sb[:, j, tsl], t1[:], t2[:])

                # V projection: [t, f] via stationary-h matmuls
                for tt in range(KC):
                    pv = psA.tile([P, FPC], F32, tag="pv", name="pv")
                    for dc in range(DC):
                        nc.tensor.matmul(pv[:], h_sb[:, dc, tt * P:(tt + 1) * P],
                                         wv_sb[:, dc], start=(dc == 0), stop=False)
                    nc.tensor.matmul(pv[:], ones1[:], bv_sb[:],
                                     start=False, stop=True)
                    nc.vector.tensor_copy(v_sb[:, tt, :, 0:64], pv[:])

            # ---------------- Phase B ----------------
            # qq (q-quarter, 512) outer x hp (head pair). Per kc: both heads'
            # scores go into one [P, 2, NQ] psum tile (row-tiled 64x128 MMs,
            # adjacent banks), ONE fused exp covers both heads, double-buffered
            # so next-kc scores overlap the current exp. 6 PSUM banks total.
            with (
                tc.tile_pool(name="peb", bufs=1) as peb,
                tc.tile_pool(name="pB", bufs=2) as pB,
                tc.tile_pool(name="psB", bufs=1, space="PSUM") as psB,
            ):
                NQQ = 4
                for qq in range(NQQ):
                    qsl = slice(qq * NQ, (qq + 1) * NQ)
                    eb = peb.tile([P, KC, NQ], F16, tag="eb", bufs=2)
                    nc.sync.dma_start(eb[:], eb_r[:, :, qsl])
                    for hp in range(FT):
                        ft = hp
                        cts = []
                        for hi in range(2):
                            ct = psB.tile([65, NQ], F32, tag=f"ct{hi}",
                                          name=f"ct{hi}")
                            cts.append(ct)
                        prev = None
                        for kc in range(KC):
                            sc = psB.tile([P, 2, NQ], F32, tag="sc", bufs=2,
                                          name="sc")
                            for hi in range(2):
                                base = 64 * hi
                                ksl = qk_sb[base:base + 64, FT + ft,
                                            kc * P:(kc + 1) * P]
                                qop = qk_sb[base:base + 64, ft, qsl]
                                nc.tensor.matmul(sc[:, hi, :], ksl, qop,
                                                 start=True, stop=True)
                            u = pB.tile([P, 2, NQ], F16, tag="u")
                            nc.scalar.activation(u[:], sc[:], AF.Exp,
                                                 bias=eshift[:])
                            u2 = pB.tile([P, 2, NQ], F16, tag="u2")
                            for hi in range(2):
                                nc.vector.tensor_mul(u2[:, hi, :], u[:, hi, :],
                                                     eb[:, kc])
                            if prev is not None:
                                for hi in range(2):
                                    h = 2 * hp + hi
                                    nc.tensor.matmul(
                                        cts[hi][:], v_sb[:, kc - 1, h, 0:65],
                                        prev[:, hi, :], start=(kc == 1),
                                        stop=False)
                            prev = u2
                        for hi in range(2):
                            h = 2 * hp + hi
                            nc.tensor.matmul(cts[hi][:], v_sb[:, KC - 1, h, 0:65],
                                             prev[:, hi, :], start=False,
                                             stop=True)
                        # finalize: evacuate ctx, reciprocal via DRAM roundtrip
                        cus = []
                        for hi in range(2):
                            cu = pB.tile([65, NQ], F32, tag=f"cu{hi}")
                            nc.vector.tensor_copy(cu[:], cts[hi][:])
                            cus.append(cu)
                        rscrs, rrecs, rscr2s, rbs = [], [], [], []
                        for hi in range(2):
                            rscr = dpool.tile([NQ], F32)
                            nc.gpsimd.dma_start(rscr[None, :], cus[hi][64:65, :])
                            rscrs.append(rscr)
                        rsqs = []
                        for hi in range(2):
                            rsq = pB.tile([32, NQ // 32], F32, tag=f"rsq{hi}")
                            nc.gpsimd.dma_start(
                                rsq[:], rscrs[hi].rearrange("(a b) -> a b", a=32))
                            rsqs.append(rsq)
                        for hi in range(2):
                            rrec = pB.tile([32, NQ // 32], F32, tag=f"rrec{hi}")
                            nc.vector.reciprocal(rrec[:], rsqs[hi][:])
                            rrecs.append(rrec)
                        for hi in range(2):
                            rscr2 = dpool.tile([NQ], F32)
                            nc.gpsimd.dma_start(
                                rscr2.rearrange("(a b) -> a b", a=32), rrecs[hi][:])
                            rscr2s.append(rscr2)
                        for hi in range(2):
                            rb = pB.tile([64, NQ], F32, tag=f"rb{hi}")
                            nc.gpsimd.dma_start(rb[:],
                                                rscr2s[hi].partition_broadcast(64))
                            rbs.append(rb)
                        for hi in range(2):
                            base = 64 * hi
                            nc.vector.tensor_mul(ctxT[base:base + 64, ft, qsl],
                                                 cus[hi][0:64, :], rbs[hi][:])

            # ---------------- Phase C ----------------
            with (
                tc.tile_pool(name="pC", bufs=3) as pC,
                tc.tile_pool(name="psC", bufs=2, space="PSUM") as psC,
            ):
                NT = NQ
                for ot in range(D // P):
                    for tq in range(S // NT):
                        tsl = slice(tq * NT, (tq + 1) * NT)
                        po = psC.tile([P, NT], F32, tag="po", name="po")
                        for fc in range(FT):
                            nc.tensor.matmul(po[:],
                                             ow_sb[:, fc, ot * P:(ot + 1) * P],
                                             ctxT[:, fc, tsl],
                                             start=(fc == 0), stop=(fc == FT - 1))
                        o_sb = pC.tile([P, NT], F16, tag="oT")
                        nc.scalar.copy(o_sb[:], po[:])
                        nc.sync.dma_start(outT.ap()[ot * P:(ot + 1) * P, tsl],
                                          o_sb[:])

    nc.compile()
    return nc


def make_core_inputs(hidden_states, attention_bias, rope_cos, rope_sin, head_mask,
                     qkv_w, qkv_b, o_w, S=2048, D=1024):
    """Host-side sharding + layout preparation. Returns list of 8 input dicts."""
    f32 = np.float32
    f16 = np.float16
    hidden_states = np.asarray(hidden_states, f32)
    attention_bias = np.asarray(attention_bias, f32)
    rope_cos = np.asarray(rope_cos, f32)
    rope_sin = np.asarray(rope_sin, f32)
    head_mask = np.asarray(head_mask, f32).reshape(-1)
    qkv_w = np.asarray(qkv_w, f32)
    qkv_b = np.asarray(qkv_b, f32)
    o_w = np.asarray(o_w, f32)

    B = hidden_states.shape[0]
    FPC = HPC * 64
    F = H * 64  # qkv feature dim (row-section size of qkv_w)

    cos_t = rope_cos[0, :, 0, :].T.astype(f32)     # [64, S]
    sin_t = rope_sin[0, :, 0, :].T.astype(f32)
    cosr = np.concatenate([cos_t, cos_t], axis=0)  # [128, S]
    sinr = np.concatenate([sin_t, sin_t], axis=0)

    # rotate_half as a signed permutation: out[c] = -in[c+32] (c%64<32),
    # +in[c-32] (c%64>=32); per 64-row head block, two blocks per 128.
    permM = np.zeros((128, 128), f32)
    for blk in (0, 64):
        for c in range(32):
            permM[blk + c + 32, blk + c] = -1.0
        for c in range(32, 64):
            permM[blk + c - 32, blk + c] = 1.0

    in_maps = []
    for c in range(8):
        b, g = divmod(c, G)
        fs = slice(g * FPC, (g + 1) * FPC)
        wq = qkv_w[F * 0:F * 1][fs]
        wk = qkv_w[F * 1:F * 2][fs]
        wv = qkv_w[F * 2:F * 3][fs].copy()
        bq = qkv_b[F * 0:F * 1][fs]
        bk = qkv_b[F * 1:F * 2][fs]
        bvv = qkv_b[F * 2:F * 3][fs].copy()
        mask = head_mask[g * HPC:(g + 1) * HPC]
        wv *= np.repeat(mask, 64)[:, None]
        bvv *= np.repeat(mask, 64)
        w4 = np.concatenate([wq.T, wk.T], axis=1)      # [D, 2*FPC]
        b4 = np.concatenate([bq, bk])
        bT = np.ascontiguousarray(attention_bias[b, 0].T)
        m = {
            "hT": np.ascontiguousarray(hidden_states[b].T).astype(f16),
            "w4": np.ascontiguousarray(w4).astype(f16),
            "b4": np.ascontiguousarray(b4),
            "wvT": np.ascontiguousarray(wv.T).astype(f16),
            "bv": np.ascontiguousarray(bvv).astype(f16),
            "cosr": np.ascontiguousarray(cosr).astype(f16),
            "sinr": np.ascontiguousarray(sinr).astype(f16),
            "permM": np.ascontiguousarray(permM).astype(f16),
            "expbT": np.exp(bT).astype(f16),
            "owT": np.ascontiguousarray(o_w[:, g * FPC:(g + 1) * FPC].T).astype(f16),
        }
        in_maps.append(m)
    return in_maps


def kernel(hidden_states, attention_bias, rope_cos, rope_sin, head_mask,
           qkv_w, qkv_b, o_w, o_b, **_unused):
    from concourse.bass_utils import run_bass_kernel_spmd

    B, S, D = hidden_states.shape
    if "nc" not in _CACHE:
        _CACHE["nc"] = build_nc(S=S, D=D)
    nc = _CACHE["nc"]

    in_maps = make_core_inputs(hidden_states, attention_bias, rope_cos, rope_sin,
                               head_mask, qkv_w, qkv_b, o_w, S=S, D=D)
    res = run_bass_kernel_spmd(nc, in_maps, list(range(8)))
    _CACHE["last_results"] = res

    o_b = np.asarray(o_b, np.float32)
    out = np.empty((B, S, D), np.float32)
    for b in range(B):
        acc = (res.results[2 * b]["outT"].astype(np.float32).T
               + res.results[2 * b + 1]["outT"].astype(np.float32).T)
        out[b] = acc + o_b[None, :]
    return out


# revision 10
# speedup vs baseline: 1.3546x; 1.0472x over previous
"""Trainium2 Bass kernel for nn_Attention_8143257993917.

Multi-head attention (packed QKV + RoPE + additive bias + softmax + head_mask
+ o_proj), B=4, S=2048, D=1024, H=16 heads, fp32 I/O.

Sharding: 8 cores = 4 batches x 2 head-groups (tensor-parallel over heads).
Core c handles batch b = c // 2 and heads g*8..g*8+8 with g = c % 2.
Each core computes a partial output (its heads' contribution through o_proj);
the host sums the two partials per batch and adds o_b.

v2 design (per core), all feature-major layouts, fp16 matmul datapath with
fp32 PSUM accumulation:
  Phase A (projections + rope):
    Q_T/K_T [f, t]: psum = w_tile.T @ hT, evacuated on ScalarE with the bias
    add folded in (activation Copy + per-partition bias), then
    q' = qpb*cos + rot(qpb)*sin where rot(qpb) comes from a single [128,128]
    signed-permutation matmul (rotate_half folds into a constant matrix and
    the bias is already inside qpb). N=1024 moving operands.
    V [t, f] via stationary-h matmuls (so V chunks are directly the PV
    stationary); ones-row matmul adds the (head_mask-folded) V bias; a
    ones-column appended to V makes PV also produce softmax denominators.
  Phase B (attention), loop qh(2) outer x hp(4), per-kc software pipeline:
    scores sc_hi[k,q] = K_chunk.T @ Q (row-tiled 64x128: h-even on array
    rows 0-63, h-odd on 64-127), exp on ScalarE (PSUM->SBUF, constant -12
    shift), u2 = exp(s)*exp(bias) on DVE (exp(bias) precomputed on host,
    fp16, loaded once per qh half), PV lags one kc. Split per-head exp
    instructions let head h0's next-kc scores matmul run while h1's exp
    occupies ScalarE, keeping ScalarE (the throughput floor: ~33.6M exp
    elems/core, ScalarE-only) saturated. PSUM: 2 scores + 2 ctx tiles = 8
    banks exactly.
    Denominator reciprocal via DRAM round-trip reshape + partition
    broadcast on the gpsimd DMA queues (as baseline).
  Phase C: o_proj out_T[o,t] = sum_fc ow.T @ ctxT, fp16 output (host sums
    partials in fp32).
"""

import sys

sys.path.insert(0, "/opt/trn_rl_repo")

import numpy as np

_CACHE = {}

H = 16
HPC = 8  # heads per core
G = 2  # head groups


def build_nc(S=2048, D=1024):
    import concourse.bass as bass
    from concourse import bacc
    import concourse.mybir as mybir
    import concourse.tile as tile

    F32 = mybir.dt.float32
    F16 = mybir.dt.float16
    AF = mybir.ActivationFunctionType

    P = 128
    DC = D // P           # contraction chunks for projections (8)
    KC = S // P           # k chunks (16)
    NQ = 512              # matmul moving free-dim max / q quarter
    NQQ = S // NQ         # q quarters (4)
    FPC = HPC * 64        # features per core (512)
    FT = FPC // P         # f-tiles per tensor (4)

    nc = bacc.Bacc("TRN2", target_bir_lowering=False, debug=False, num_devices=8)

    hT = nc.dram_tensor("hT", [D, S], F16, kind="ExternalInput")
    w4 = nc.dram_tensor("w4", [D, 2 * FPC], F16, kind="ExternalInput")
    b4 = nc.dram_tensor("b4", [2 * FPC], F32, kind="ExternalInput")
    wvT = nc.dram_tensor("wvT", [D, FPC], F16, kind="ExternalInput")
    bv = nc.dram_tensor("bv", [FPC], F16, kind="ExternalInput")
    cosr = nc.dram_tensor("cosr", [P, S], F16, kind="ExternalInput")
    sinr = nc.dram_tensor("sinr", [P, S], F16, kind="ExternalInput")
    permM = nc.dram_tensor("permM", [P, P], F16, kind="ExternalInput")
    expbT = nc.dram_tensor("expbT", [S, S], F16, kind="ExternalInput")
    owT = nc.dram_tensor("owT", [FPC, D], F16, kind="ExternalInput")
    outT = nc.dram_tensor("outT", [D, S], F16, kind="ExternalOutput")

    hT_r = hT.ap().rearrange("(o p) t -> p o t", p=P)
    w4_r = w4.ap().rearrange("(o p) f -> p o f", p=P)
    wv_r = wvT.ap().rearrange("(o p) f -> p o f", p=P)
    ow_r = owT.ap().rearrange("(o p) f -> p o f", p=P)
    b4_r = b4.ap().rearrange("(o p) -> p o", p=P)
    eb_r = expbT.ap().rearrange("(kc p) q -> p kc q", p=P)

    with tile.TileContext(nc) as tc:
        with (
            tc.tile_pool(name="cst", bufs=1) as cst,
            tc.tile_pool(name="pAB", bufs=1) as pAB,
            tc.tile_pool(name="pA", bufs=1) as pA,
            tc.tile_pool(name="dram", bufs=4, space="DRAM") as dpool,
        ):
            ones1 = cst.tile([1, P], F16)
            nc.vector.memset(ones1[:], 1.0)
            b4_sb = cst.tile([P, 2 * FPC // P], F32)
            nc.sync.dma_start(b4_sb[:], b4_r)
            bv_sb = cst.tile([1, FPC], F16)
            nc.sync.dma_start(bv_sb[:], bv.ap()[None, :])
            eshift = cst.tile([P, 1], F32)
            nc.vector.memset(eshift[:], -12.0)
            permM_sb = cst.tile([P, P], F16)
            nc.sync.dma_start(permM_sb[:], permM.ap())

            # persistent phase-A products
            qk_sb = pAB.tile([P, 2 * FT, S], F16)      # Q ft 0..3, K ft 4..7
            v_sb = pAB.tile([P, KC, HPC, 66], F16)     # col 64 = ones
            nc.vector.memset(v_sb[:, :, :, 64:65], 1.0)
            ctxT = pAB.tile([P, FT, S], F16)
            ow_sb = pAB.tile([P, FT, D], F16)
            nc.sync.dma_start(ow_sb[:], ow_r)

            # phase-A working set (resident until the projection weave ends)
            h_sb = pA.tile([P, DC, S], F16)
            for dc in range(DC):
                nc.sync.dma_start(h_sb[:, dc], hT_r[:, dc])
            cos_sb = pA.tile([P, S], F16)
            nc.sync.dma_start(cos_sb[:], cosr.ap())
            sin_sb = pA.tile([P, S], F16)
            nc.sync.dma_start(sin_sb[:], sinr.ap())
            wv_sb = pA.tile([P, DC, FPC], F16)
            nc.sync.dma_start(wv_sb[:], wv_r)

            def emit_qk_chunk(psum_pool, sb_pool, wa, j, t4):
                """Project + rope one [128-feature, 512-t] chunk of Q or K."""
                tsl = slice(t4 * NQ, (t4 + 1) * NQ)
                pa = psum_pool.tile([P, NQ], F32, tag="paW", name="paW")
                for dc in range(DC):
                    nc.tensor.matmul(pa[:], wa[:, dc], h_sb[:, dc, tsl],
                                     start=(dc == 0), stop=(dc == DC - 1))
                qpb = sb_pool.tile([P, NQ], F16, tag="qpbW")
                nc.vector.tensor_scalar_add(qpb[:], pa[:], b4_sb[:, j:j + 1])
                pr = psum_pool.tile([P, NQ], F32, tag="prW", name="prW")
                nc.tensor.matmul(pr[:], permM_sb[:], qpb[:],
                                 start=True, stop=True)
                t1 = sb_pool.tile([P, NQ], F16, tag="t1W")
                nc.vector.tensor_mul(t1[:], qpb[:], cos_sb[:, tsl])
                t2 = sb_pool.tile([P, NQ], F16, tag="t2W")
                nc.vector.tensor_mul(t2[:], pr[:], sin_sb[:, tsl])
                nc.vector.tensor_add(qk_sb[:, j, tsl], t1[:], t2[:])

            def load_w_tile(sb_pool, j, tag="wW"):
                wa = sb_pool.tile([P, DC, P], F16, tag=tag)
                nc.sync.dma_start(wa[:], w4_r[:, :, j * P:(j + 1) * P])
                return wa

            # ---------------- Preamble: V (all) + K ft0 + Q ft0 ------------
            with (
                tc.tile_pool(name="pAw", bufs=2) as pAw,
                tc.tile_pool(name="psP", bufs=1, space="PSUM") as psP,
            ):
                wK = load_w_tile(pAw, FT + 0, tag="wK")
                wQ = load_w_tile(pAw, 0, tag="wQ")
                for tt in range(KC):
                    pv = psP.tile([P, FPC], F32, tag="pv", name="pv", bufs=2)
                    for dc in range(DC):
                        nc.tensor.matmul(pv[:], h_sb[:, dc, tt * P:(tt + 1) * P],
                                         wv_sb[:, dc], start=(dc == 0), stop=False)
                    nc.tensor.matmul(pv[:], ones1[:], bv_sb[:],
                                     start=False, stop=True)
                    nc.vector.tensor_copy(v_sb[:, tt, :, 0:64], pv[:])
                for t4 in range(NQQ):
                    emit_qk_chunk(psP, pAw, wK, FT + 0, t4)
                for t4 in range(NQQ):
                    emit_qk_chunk(psP, pAw, wQ, 0, t4)

            # ---------------- Phase B with projection/o_proj weave ---------
            # Remaining A work: (K ft, Q ft) for hp 1..3, woven one chunk per
            # kc-block during the previous hp's stream. o_proj for quarter qq
            # woven into quarter qq+1. PSUM: sc 2x2 + ct 2 = 6 banks (B) +
            # 2 banks (weave pa/pr, later o_proj po).
            with (
                tc.tile_pool(name="peb", bufs=1) as peb,
                tc.tile_pool(name="pB", bufs=2) as pB,
                tc.tile_pool(name="psB", bufs=1, space="PSUM") as psB,
                tc.tile_pool(name="pW", bufs=2) as pW,
            ):
                # A-weave chunk list: (j, t4) in the order hp1-needs, hp2, hp3
                aw = []
                for hp in range(1, FT):
                    for j in (FT + hp, hp):
                        aw.append((j, None))           # weight-load marker
                        for t4 in range(NQQ):
                            aw.append((j, t4))
                aw_weights = {}
                awi = 0

                def weave_a():
                    nonlocal awi
                    # emit up to 2 items per call (weight loads are free)
                    budget = 1
                    while budget > 0 and awi < len(aw):
                        j, t4 = aw[awi]
                        if t4 is None:
                            aw_weights[j] = load_w_tile(pW, j, tag=f"wW{j % 2}")
                        else:
                            emit_qk_chunk(psW, pW, aw_weights[j], j, t4)
                            budget -= 1
                        awi += 1

                co = []                                 # pending o_proj chunks
                def weave_c():
                    if co:
                        co.pop(0)()

                def emit_oproj(qq):
                    qsl = slice(qq * NQ, (qq + 1) * NQ)
                    for ot in range(D // P):
                        def emit(ot=ot, qsl=qsl):
                            po = psW.tile([P, NQ], F32, tag="prW", name="po")
                            for fc in range(FT):
                                nc.tensor.matmul(
                                    po[:], ow_sb[:, fc, ot * P:(ot + 1) * P],
                                    ctxT[:, fc, qsl],
                                    start=(fc == 0), stop=(fc == FT - 1))
                            o_sb = pW.tile([P, NQ], F16, tag="oT")
                            nc.vector.tensor_copy(o_sb[:], po[:])
                            nc.sync.dma_start(
                                outT.ap()[ot * P:(ot + 1) * P, qsl], o_sb[:])
                        co.append(emit)

                with tc.tile_pool(name="psW", bufs=1, space="PSUM") as psW:
                    for qq in range(NQQ):
                        qsl = slice(qq * NQ, (qq + 1) * NQ)
                        eb = peb.tile([P, KC, NQ], F16, tag="eb", bufs=2)
                        nc.sync.dma_start(eb[:], eb_r[:, :, qsl])
                        for hp in range(FT):
                            ft = hp
                            cts = []
                            for hi in range(2):
                                ct = psB.tile([65, NQ], F32, tag=f"ct{hi}",
                                              name=f"ct{hi}")
                                cts.append(ct)
                            prev = None
                            for kc in range(KC):
                                sc = psB.tile([P, 2, NQ], F32, tag="sc",
                                              bufs=2, name="sc")
                                for hi in range(2):
                                    base = 64 * hi
                                    ksl = qk_sb[base:base + 64, FT + ft,
                                                kc * P:(kc + 1) * P]
                                    qop = qk_sb[base:base + 64, ft, qsl]
                                    nc.tensor.matmul(sc[:, hi, :], ksl, qop,
                                                     start=True, stop=True)
                                u = pB.tile([P, 2, NQ], F16, tag="u")
                                nc.scalar.activation(u[:], sc[:], AF.Exp,
                                                     bias=eshift[:])
                                u2 = pB.tile([P, 2, NQ], F16, tag="u2")
                                for hi in range(2):
                                    nc.vector.tensor_mul(u2[:, hi, :],
                                                         u[:, hi, :], eb[:, kc])
                                if prev is not None:
                                    for hi in range(2):
                                        h = 2 * hp + hi
                                        nc.tensor.matmul(
                                            cts[hi][:],
                                            v_sb[:, kc - 1, h, 0:65],
                                            prev[:, hi, :], start=(kc == 1),
                                            stop=False)
                                prev = u2
                                # weave one deferred chunk every other block
                                if kc % 2 == 0:
                                    weave_a()
                                elif kc % 4 == 1:
                                    weave_c()
                            for hi in range(2):
                                h = 2 * hp + hi
                                nc.tensor.matmul(cts[hi][:],
                                                 v_sb[:, KC - 1, h, 0:65],
                                                 prev[:, hi, :], start=False,
                                                 stop=True)
                            # finalize
                            cus = []
                            for hi in range(2):
                                cu = pB.tile([65, NQ], F32, tag=f"cu{hi}")
                                nc.vector.tensor_copy(cu[:], cts[hi][:])
                                cus.append(cu)
                            rscrs, rrecs, rscr2s, rbs = [], [], [], []
                            for hi in range(2):
                                rscr = dpool.tile([NQ], F32)
                                nc.gpsimd.dma_start(rscr[None, :],
                                                    cus[hi][64:65, :])
                                rscrs.append(rscr)
                            rsqs = []
                            for hi in range(2):
                                rsq = pB.tile([32, NQ // 32], F32,
                                              tag=f"rsq{hi}")
                                nc.gpsimd.dma_start(
                                    rsq[:],
                                    rscrs[hi].rearrange("(a b) -> a b", a=32))
                                rsqs.append(rsq)
                            for hi in range(2):
                                rrec = pB.tile([32, NQ // 32], F32,
                                               tag=f"rrec{hi}")
                                nc.vector.reciprocal(rrec[:], rsqs[hi][:])
                                rrecs.append(rrec)
                            for hi in range(2):
                                rscr2 = dpool.tile([NQ], F32)
                                nc.gpsimd.dma_start(
                                    rscr2.rearrange("(a b) -> a b", a=32),
                                    rrecs[hi][:])
                                rscr2s.append(rscr2)
                            for hi in range(2):
                                rb = pB.tile([64, NQ], F32, tag=f"rb{hi}")
                                nc.gpsimd.dma_start(
                                    rb[:], rscr2s[hi].partition_broadcast(64))
                                rbs.append(rb)
                            for hi in range(2):
                                base = 64 * hi
                                nc.vector.tensor_mul(
                                    ctxT[base:base + 64, ft, qsl],
                                    cus[hi][0:64, :], rbs[hi][:])
                        # queue o_proj for this finished quarter
                        emit_oproj(qq)
                    # drain any remaining weave work (o_proj tail)
                    while awi < len(aw):
                        weave_a()
                    while co:
                        weave_c()

    nc.compile()
    return nc


def make_core_inputs(hidden_states, attention_bias, rope_cos, rope_sin, head_mask,
                     qkv_w, qkv_b, o_w, S=2048, D=1024):
    """Host-side sharding + layout preparation. Returns list of 8 input dicts."""
    f32 = np.float32
    f16 = np.float16
    hidden_states = np.asarray(hidden_states, f32)
    attention_bias = np.asarray(attention_bias, f32)
    rope_cos = np.asarray(rope_cos, f32)
    rope_sin = np.asarray(rope_sin, f32)
    head_mask = np.asarray(head_mask, f32).reshape(-1)
    qkv_w = np.asarray(qkv_w, f32)
    qkv_b = np.asarray(qkv_b, f32)
    o_w = np.asarray(o_w, f32)

    B = hidden_states.shape[0]
    FPC = HPC * 64
    F = H * 64  # qkv feature dim (row-section size of qkv_w)

    cos_t = rope_cos[0, :, 0, :].T.astype(f32)     # [64, S]
    sin_t = rope_sin[0, :, 0, :].T.astype(f32)
    cosr = np.concatenate([cos_t, cos_t], axis=0)  # [128, S]
    sinr = np.concatenate([sin_t, sin_t], axis=0)

    # rotate_half as a signed permutation: out[c] = -in[c+32] (c%64<32),
    # +in[c-32] (c%64>=32); per 64-row head block, two blocks per 128.
    permM = np.zeros((128, 128), f32)
    for blk in (0, 64):
        for c in range(32):
            permM[blk + c + 32, blk + c] = -1.0
        for c in range(32, 64):
            permM[blk + c - 32, blk + c] = 1.0

    in_maps = []
    for c in range(8):
        b, g = divmod(c, G)
        fs = slice(g * FPC, (g + 1) * FPC)
        wq = qkv_w[F * 0:F * 1][fs]
        wk = qkv_w[F * 1:F * 2][fs]
        wv = qkv_w[F * 2:F * 3][fs].copy()
        bq = qkv_b[F * 0:F * 1][fs]
        bk = qkv_b[F * 1:F * 2][fs]
        bvv = qkv_b[F * 2:F * 3][fs].copy()
        mask = head_mask[g * HPC:(g + 1) * HPC]
        wv *= np.repeat(mask, 64)[:, None]
        bvv *= np.repeat(mask, 64)
        w4 = np.concatenate([wq.T, wk.T], axis=1)      # [D, 2*FPC]
        b4 = np.concatenate([bq, bk])
        bT = np.ascontiguousarray(attention_bias[b, 0].T)
        m = {
            "hT": np.ascontiguousarray(hidden_states[b].T).astype(f16),
            "w4": np.ascontiguousarray(w4).astype(f16),
            "b4": np.ascontiguousarray(b4),
            "wvT": np.ascontiguousarray(wv.T).astype(f16),
            "bv": np.ascontiguousarray(bvv).astype(f16),
            "cosr": np.ascontiguousarray(cosr).astype(f16),
            "sinr": np.ascontiguousarray(sinr).astype(f16),
            "permM": np.ascontiguousarray(permM).astype(f16),
            "expbT": np.exp(bT).astype(f16),
            "owT": np.ascontiguousarray(o_w[:, g * FPC:(g + 1) * FPC].T).astype(f16),
        }
        in_maps.append(m)
    return in_maps


def kernel(hidden_states, attention_bias, rope_cos, rope_sin, head_mask,
           qkv_w, qkv_b, o_w, o_b, **_unused):
    from concourse.bass_utils import run_bass_kernel_spmd

    B, S, D = hidden_states.shape
    if "nc" not in _CACHE:
        _CACHE["nc"] = build_nc(S=S, D=D)
    nc = _CACHE["nc"]

    in_maps = make_core_inputs(hidden_states, attention_bias, rope_cos, rope_sin,
                               head_mask, qkv_w, qkv_b, o_w, S=S, D=D)
    res = run_bass_kernel_spmd(nc, in_maps, list(range(8)))
    _CACHE["last_results"] = res

    o_b = np.asarray(o_b, np.float32)
    out = np.empty((B, S, D), np.float32)
    for b in range(B):
        acc = (res.results[2 * b]["outT"].astype(np.float32).T
               + res.results[2 * b + 1]["outT"].astype(np.float32).T)
        out[b] = acc + o_b[None, :]
    return out
